# revision 1
# baseline (speedup 1.0000x reference)
"""CombinePatches (3D col2im fold + overlap-count normalize) on 8 TRN2 NeuronCores.

Decomposition (validated numerically against the reference):
  out[b, 2q+kd, 2s+kh, 2u+kw, c] (+)= patches[b, q, s, u, kd, kh, kw, c], then
  out /= cnt, cnt = cd(d)*ch(h)*cw(w) separable overlap counts.

Sharding: 8 cores = B(2) x D-chunks(4). Each core computes 16 output d-rows from
9 od-slices of patches (1 halo slice, zero-padded at global edges by the host).

Per core, per output row d (r=d%2, q=d//2):
  - DVE w-fold: T[s, j, w, c] = A[s, floor(w/2), j, ...] + A[s, floor(w/2)-1, ...]
    done for A = slice q (kd=r) and B = slice q-1 (kd=r+2), with the ow dim
    pre-split into two halves on partitions (p = uhalf*64 + s) so each DVE op
    uses all 128 lanes.
  - TensorE h-fold: O[h, (w,c)] = sum_j Mh_j^T @ T_j accumulated in PSUM over
    (j x {A,B} x {w-half}) = 16 float32r matmuls; 0.25*rh(h) baked into Mh
    (0.25 = interior rd * interior rw).
  - ScalarE eviction: PSUM -> SBUF copy, then DMA store on the scalar ring.
Host fixes the global d-edge rows and w-edge columns by x2 after gather.
"""
import sys

for _p in ("/opt/trn_rl_repo", "/opt/trn_rl_repo/pypackages"):
    if _p not in sys.path:
        sys.path.insert(0, _p)

from contextlib import ExitStack

import numpy as np

import concourse.bass as bass
import concourse.tile as tile
from concourse import bacc, mybir
from concourse import bass_utils

B, D, H, W, C = 2, 64, 128, 128, 4
od, oh, ow = 31, 63, 63
NS, X = 9, 33       # od-slices per core (incl 1 halo), padded u-slots per half
RPC = 16            # output d-rows per core
MM_DT = mybir.dt.float32r

_cache = {}


def _build():
    nc = bacc.Bacc(
        "TRN2",
        target_bir_lowering=False,
        debug=False,
        enable_asserts=False,
        num_devices=8,
    )
    # flat pp: [half-slice k=0 (kd 2,3 only)] + [7 full slices] + [half k=8 (kd 0,1)]
    HALF_F, FULL_F = X * 128, X * 256
    PP_TOTAL = 128 * (2 * HALF_F + 7 * FULL_F)
    pp_d = nc.dram_tensor(
        "pp", [PP_TOTAL], mybir.dt.float32, kind="ExternalInput"
    ).ap()
    wm_d = nc.dram_tensor("wm", [128, 1024], MM_DT, kind="ExternalInput").ap()
    out_d = nc.dram_tensor(
        "out", [RPC, H, W, C], mybir.dt.float32, kind="ExternalOutput"
    ).ap()

    with ExitStack() as ctx:
        tc = ctx.enter_context(tile.TileContext(nc))
        const_pool = ctx.enter_context(tc.tile_pool(name="const", bufs=1))
        slice_pool = ctx.enter_context(tc.tile_pool(name="slice", bufs=3))
        t_pool = ctx.enter_context(tc.tile_pool(name="tt", bufs=4))
        ev_pool = ctx.enter_context(tc.tile_pool(name="ev", bufs=3))
        psum_pool = ctx.enter_context(tc.tile_pool(name="ps", bufs=4, space="PSUM"))

        # constants go on the scalar-engine HWDGE ring so the sync ring is
        # purely slice loads (HWDGE rings are FIFO per issuing engine).
        wm_sb = const_pool.tile([128, 1024], MM_DT)
        nc.scalar.dma_start(wm_sb[:], wm_d[:])

        def slice_region(k):
            """(flat offset, free width, n_kd, kd_base) of slice k in pp."""
            if k == 0:
                return 0, HALF_F, 2, 2
            if k == NS - 1:
                return 128 * (HALF_F + 7 * FULL_F), HALF_F, 2, 0
            return 128 * (HALF_F + (k - 1) * FULL_F), FULL_F, 4, 0

        tiles = {}
        for k in range(NS):
            off, fw, nkd, kdb = slice_region(k)
            t = slice_pool.tile([128, fw], mybir.dt.float32, tag="slice")
            src = pp_d[off : off + 128 * fw].rearrange("(p f) -> p f", f=fw)
            nc.sync.dma_start(t[:], src)
            tiles[k] = (t, nkd, kdb)
            if k == 0:
                continue
            for rr in range(2):
                d_loc = 2 * (k - 1) + rr
                TA = t_pool.tile([128, 1024], MM_DT, tag="T")
                TB = t_pool.tile([128, 1024], MM_DT, tag="T")
                for T, (tk, t_nkd, t_kdb), kd in (
                    (TA, tiles[k], rr),
                    (TB, tiles[k - 1], rr + 2),
                ):
                    v = tk[:].rearrange(
                        "p (x kd j v c) -> p x kd j v c", x=X, kd=t_nkd, j=4, v=4, c=4
                    )
                    ki = kd - t_kdb
                    t1 = v[:, 1:33, ki, :, 0:2, :].rearrange("p m j t c -> p j m t c")
                    t2 = v[:, 0:32, ki, :, 2:4, :].rearrange("p m j t c -> p j m t c")
                    To = T[:].rearrange("p (j m t c) -> p j m t c", j=4, m=32, t=2, c=4)
                    nc.vector.tensor_add(To, t1, t2)
                ps = psum_pool.tile([128, 512], mybir.dt.float32, tag="ps")
                for half in range(2):
                    outseg = ps[:, half * 256 : (half + 1) * 256]
                    n = 0
                    for j in range(4):
                        # K=128 with zero-padded block-diagonal weights keeps
                        # every matmul at tile_position (0,0): mixing PE tile
                        # positions in one NEFF hangs at runtime.
                        lhsT = wm_sb[:, 512 * half + j * 128 : 512 * half + (j + 1) * 128]
                        for T in (TA, TB):
                            rhs = T[:, j * 256 : (j + 1) * 256]
                            nc.tensor.matmul(
                                outseg, lhsT, rhs, start=(n == 0), stop=(n == 7)
                            )
                            n += 1
                # evict on ScalarE: evictions wait on matmuls, and in the DVE
                # FIFO they would delay later w-folds, which gate slice loads
                # via slot release. rw's interior 0.5 is folded into wm; the
                # host rescales the 4 global w-edge columns.
                ev = ev_pool.tile([128, 512], mybir.dt.float32, tag="ev")
                nc.scalar.copy(ev[:], ps[:])
                # stores on the scalar ring: a store waiting on eviction must
                # not head-of-line-block the next slice load on the sync ring
                nc.scalar.dma_start(out_d[d_loc].rearrange("h w c -> h (w c)"), ev[:])
    nc.compile()
    return nc


def _host_tables():
    rh = np.where(
        (np.arange(H) < 2) | (np.arange(H) >= H - 2), 1.0, 0.5
    ).astype(np.float32)
    # [half*64+s, whalf*512 + j*128 + h], block-diagonal in (half, whalf).
    # 0.25 = interior rd (0.5) * interior rw (0.5); host rescales d/w edges.
    wm = np.zeros((128, 1024), np.float32)
    s_idx = np.arange(oh)
    for j in range(4):
        h = 2 * s_idx + j
        wm[s_idx, j * 128 + h] = 0.25 * rh[h]
        wm[64 + s_idx, 512 + j * 128 + h] = 0.25 * rh[h]
    return wm


def _shard_inputs(patches):
    """Build per-core flat patch blocks: half k=0 (kd 2,3) + 7 full + half k=8
    (kd 0,1), each region [128 partitions x freewidth] flattened p-major."""
    P5 = np.ascontiguousarray(patches).reshape(B, od, oh, ow, 256)
    # q-slot k = q+1 for q in [-1, 32); u-slot x = u+1 for u in [-1, 65)
    Pu = np.zeros((B, od + 2, 64, 66, 256), np.float32)
    Pu[:, 1 : od + 1, 0:oh, 1 : ow + 1, :] = P5
    pps = []
    for core in range(8):
        b, kc = core // 4, core % 4
        s0 = 8 * kc  # = qbase + 1
        # [NS, 2(uhalf), 64(s), X, 256]
        pp = np.stack(
            [Pu[b, s0 : s0 + NS, :, 0:X, :], Pu[b, s0 : s0 + NS, :, 32 : 32 + X, :]],
            axis=1,
        )
        parts = [
            np.ascontiguousarray(pp[0, :, :, :, 128:256]).reshape(-1),  # kd 2,3
            np.ascontiguousarray(pp[1 : NS - 1]).reshape(-1),
            np.ascontiguousarray(pp[NS - 1, :, :, :, 0:128]).reshape(-1),  # kd 0,1
        ]
        pps.append(np.concatenate(parts))
    return pps


def _run(patches, trace=False):
    if "nc" not in _cache:
        _cache["nc"] = _build()
        _cache["tables"] = _host_tables()
    nc = _cache["nc"]
    wm = _cache["tables"]
    pps = _shard_inputs(np.asarray(patches, dtype=np.float32))
    in_maps = [{"pp": pps[core], "wm": wm} for core in range(8)]
    res = bass_utils.run_bass_kernel_spmd(
        nc, in_maps, core_ids=list(range(8)), trace=trace
    )
    out = np.zeros((B, D, H, W, C), np.float32)
    for core in range(8):
        b, kc = core // 4, core % 4
        out[b, RPC * kc : RPC * (kc + 1)] = res.results[core]["out"]
    out[:, [0, 1, D - 2, D - 1]] *= 2.0
    out[:, :, :, [0, 1, W - 2, W - 1], :] *= 2.0
    return out, res


def kernel(patches, inputs):
    out, _ = _run(patches)
    return out



# revision 5
# speedup vs baseline: 1.7422x; 1.7422x over previous
"""CombinePatches (3D col2im fold + overlap-count normalize) on 8 TRN2 NeuronCores.

Decomposition (validated numerically against the reference):
  out[b, 2q+kd, 2s+kh, 2u+kw, c] (+)= patches[b, q, s, u, kd, kh, kw, c], then
  out /= cnt, cnt = cd(d)*ch(h)*cw(w) separable overlap counts.

Sharding: 8 cores = B(2) x D-chunks(4). Each core computes 16 output d-rows from
9 od-slices of patches (1 halo slice, zero-padded at global edges by the host).

v2 design (HBM-bound problem; measured baseline streamed fp32 at the 358 GB/s
per-core roofline, so the wins are fewer bytes + shorter tail):
  - patch stream in bf16 (halves input DMA; end-to-end rel err ~3e-3, validated
    against the reference in numpy with exact-layout simulation).
  - SBUF slice layout [p = kh_lo*63 + s, (kh_hi, kd, kw_pair, x=u+1, kw_lo, c)]:
    the w-fold becomes a fully contiguous 512-elem DVE add (no strided gather),
    and the h-fold contracts K=(kh_lo, s)=126 packed partitions, so each output
    row needs only 4 matmuls of N=512 (PE max) instead of 16 of N=256.
  - 0.25*rh(h) (interior rd * interior rw * exact rh) baked into the weights;
    host rescales the 4 global d-edge rows and w-edge columns by 2 after gather.
  - evict/store alternate between ACT and Pool engines so the drain never
    serializes on one engine; slice loads stream on the sync ring from 9
    statically allocated SBUF tiles (no pool-slot release in the DMA path).
"""
import sys

for _p in ("/opt/trn_rl_repo", "/opt/trn_rl_repo/pypackages"):
    if _p not in sys.path:
        sys.path.insert(0, _p)

from contextlib import ExitStack

import numpy as np
import ml_dtypes

import concourse.bass as bass
import concourse.tile as tile
from concourse import bacc, mybir
from concourse import bass_utils

B, D, H, W, C = 2, 64, 128, 128, 4
od, oh, ow = 31, 63, 63
NS = 9              # od-slices per core (incl 1 halo)
RPC = 16            # output d-rows per core
X = 65              # x slots (x = u+1; pads at x=0 and x=64)
WFULL = 2 * 4 * 2 * X * 8   # (pr, kd, vg, x, vh, c) free width, full slice
WHALF = WFULL // 2          # half slices carry only 2 kd values
BF = mybir.dt.bfloat16

_cache = {}


def _slice_width(k):
    return WHALF if k in (0, NS - 1) else WFULL


_OFFS = np.concatenate([[0], np.cumsum([126 * _slice_width(k) for k in range(NS)])])
PP_TOTAL = int(_OFFS[-1])


def _build():
    nc = bacc.Bacc(
        "TRN2",
        target_bir_lowering=False,
        debug=False,
        enable_asserts=False,
        num_devices=8,
    )
    pp_d = nc.dram_tensor("pp", [PP_TOTAL], BF, kind="ExternalInput").ap()
    wm_d = nc.dram_tensor("wm", [126, 256], BF, kind="ExternalInput").ap()
    out_d = nc.dram_tensor("out", [RPC, H, W * C], BF, kind="ExternalOutput").ap()

    with ExitStack() as ctx:
        tc = ctx.enter_context(tile.TileContext(nc))
        const_pool = ctx.enter_context(tc.tile_pool(name="const", bufs=1))
        slice_pool = ctx.enter_context(tc.tile_pool(name="slice", bufs=NS))
        t_pool = ctx.enter_context(tc.tile_pool(name="tt", bufs=8))
        ev_pool = ctx.enter_context(tc.tile_pool(name="ev", bufs=4))
        psum_pool = ctx.enter_context(tc.tile_pool(name="ps", bufs=4, space="PSUM"))

        # weights on the scalar ring so the sync ring is purely slice loads
        wm_sb = const_pool.tile([126, 256], BF)
        nc.scalar.dma_start(wm_sb[:], wm_d[:])

        # statically allocated slice tiles: all 9 loads issue immediately and
        # stream back-to-back on the sync ring at full HBM bandwidth
        tiles = []
        for k in range(NS):
            w = _slice_width(k)
            t = slice_pool.tile([126, w], BF, tag="slice")
            src = pp_d[int(_OFFS[k]) : int(_OFFS[k]) + 126 * w].rearrange(
                "(p f) -> p f", f=w
            )
            nc.sync.dma_start(t[:], src)
            tiles.append(t)

        for k in range(1, NS):
            nkd_A = 2 if k == NS - 1 else 4
            nkd_B = 2 if k - 1 == 0 else 4
            for rr in range(2):
                d_loc = 2 * (k - 1) + rr
                # w-fold: T[p, (w_half? no: a,t,c)] = P[u=a, kw=t] + P[u=a-1, kw=t+2]
                # both operands are contiguous 512-elem runs in the slice tile.
                Ts = []
                for pr in range(2):
                    for (tk, nkd, ki) in (
                        (tiles[k], nkd_A, rr),
                        (tiles[k - 1], nkd_B, rr + 2 if k - 1 > 0 else rr),
                    ):
                        base = pr * (1040 * nkd) + ki * 1040
                        T = t_pool.tile([126, 512], BF, tag="T")
                        nc.vector.tensor_add(
                            T[:],
                            tk[:, base + 8 : base + 520],
                            tk[:, base + 520 : base + 1032],
                        )
                        Ts.append((pr, T))
                # h-fold: K=(kh_lo, s)=126 packed; 4 matmuls of N=512 per row
                ps = psum_pool.tile([128, 512], mybir.dt.float32, tag="ps")
                for n, (pr, T) in enumerate(Ts):
                    nc.tensor.matmul(
                        ps[:],
                        wm_sb[:, pr * 128 : (pr + 1) * 128],
                        T[:],
                        start=(n == 0),
                        stop=(n == 3),
                    )
                # evict + store, alternating ACT/Pool so the tail never
                # serializes on a single engine's FIFO
                ev = ev_pool.tile([128, 512], BF, tag="ev")
                if d_loc % 2 == 0:
                    nc.scalar.copy(ev[:], ps[:])
                    nc.scalar.dma_start(out_d[d_loc], ev[:])
                else:
                    nc.vector.tensor_copy(ev[:], ps[:])
                    nc.gpsimd.dma_start(out_d[d_loc], ev[:])
    nc.compile()
    return nc


def _host_wm():
    rh = np.where(
        (np.arange(H) < 2) | (np.arange(H) >= H - 2), 1.0, 0.5
    ).astype(np.float32)
    wm = np.zeros((126, 256), np.float32)
    jj = np.arange(2)[:, None]
    s = np.arange(63)[None, :]
    for pr in range(2):
        h = (2 * s + 2 * pr + jj).ravel()
        wm[np.arange(126), pr * 128 + h] = 0.25 * rh[h]
    return wm.astype(ml_dtypes.bfloat16)


def _shard_inputs(patches):
    """Per-core flat bf16 patch blocks, 9 slices each in layout
    [p = kh_lo*63 + s, (kh_hi, kd, kw_pair, x=u+1, kw_lo, c)]."""
    P5 = np.asarray(patches, np.float32).reshape(B, od, oh, ow, 4, 4, 4, 4)
    P5 = P5.astype(ml_dtypes.bfloat16)
    pps = []
    for core in range(8):
        b, kc = core // 4, core % 4
        parts = []
        for k in range(NS):
            q = 8 * kc - 1 + k
            kdl = slice(2, 4) if k == 0 else slice(0, 2) if k == NS - 1 else slice(0, 4)
            nkd = 2 if k in (0, NS - 1) else 4
            arr = np.zeros((2, 63, 2, nkd, 2, X, 2, 4), ml_dtypes.bfloat16)
            if 0 <= q < od:
                src = P5[b, q, :, :, kdl]                      # s,u,kd',kh,kw,c
                s6 = src.reshape(63, 63, nkd, 2, 2, 2, 2, 4)   # s,u,kd,pr,jj,vg,vh,c
                arr[:, :, :, :, :, 1:64] = s6.transpose(4, 0, 3, 2, 5, 1, 6, 7)
            parts.append(arr.reshape(-1))
        pps.append(np.concatenate(parts))
    return pps


def _run(patches, trace=False):
    if "nc" not in _cache:
        _cache["nc"] = _build()
        _cache["wm"] = _host_wm()
    nc = _cache["nc"]
    wm = _cache["wm"]
    pps = _shard_inputs(patches)
    in_maps = [{"pp": pps[core], "wm": wm} for core in range(8)]
    res = bass_utils.run_bass_kernel_spmd(
        nc, in_maps, core_ids=list(range(8)), trace=trace
    )
    out = np.zeros((B, D, H, W, C), np.float32)
    for core in range(8):
        b, kc = core // 4, core % 4
        r = np.asarray(res.results[core]["out"]).astype(np.float32)
        out[b, RPC * kc : RPC * (kc + 1)] = r.reshape(RPC, H, W, C)
    out[:, [0, 1, D - 2, D - 1]] *= 2.0
    out[:, :, :, [0, 1, W - 2, W - 1], :] *= 2.0
    return out, res


def kernel(patches, inputs):
    out, _ = _run(patches)
    return out
